# revision 2
# baseline (speedup 1.0000x reference)
"""Chessboard rearrangement kernel for Trainium2.

Input  [64, 256, 256, 16] f32 -> output [64, 8, 8, 16384] f32 where
out[b, i, j] = inputs[b, i*32:(i+1)*32, j*32:(j+1)*32, :].reshape(-1).

Pure data movement (memory-bound): the permutation granule is one
2 KB chunk (32 W-pixels x 16 channels). Implemented as direct
DRAM->DRAM DMA with 3D access patterns - per (sample, cell-row) block,
one DMA reads 512 KB linearly and scatter-writes 2 KB chunks into the 8
output cells. Batch axis is sharded 8-way across NeuronCores (8 samples
per core, 32 MiB in + 32 MiB out each; ~187 us HBM roofline at
358 GB/s). DMAs are split across both HWDGE queues (SP + ACT)
INTERLEAVED even/odd by (sample, cell-row) index: keeping the two
queues' concurrent HBM footprints adjacent (same rows/banks window)
measured 189.0 us steady-state vs 199.4 us for the
first-half/second-half split, vs 223 us for 8x 4 MiB DMAs with
globally-strided reads, vs ~198 us for a single queue. Linear-read +
scattered-write beats the gather-read + linear-write dual; through-SBUF
staging doubles SDMA work and loses. Per-DMA `.then_inc` is mandatory
(walrus: "DGE must have sync info").
"""

import sys

sys.path.insert(0, "/opt/trn_rl_repo")

import numpy as np

import concourse.bass as bass
import concourse.mybir as mybir
from concourse.bass_utils import run_bass_kernel_spmd

B, H, W, C = 64, 256, 256, 16
N_CORES = 8
B_PER = B // N_CORES          # 8 samples per core
HC, WC = H // 8, W // 8       # 32, 32 per-cell spatial dims
CELL = HC * WC * C            # 16384 elements per output cell
SAMPLE = H * W * C            # 1048576 elements per sample
ROWBLK = HC * W * C           # 131072 elements per input cell-row block
CHUNK = WC * C                # 512 contiguous elements (2 KB)

_cached = {}


def _build(reps: int = 1):
    if reps in _cached:
        return _cached[reps]
    nc = bass.Bass()
    x = nc.declare_dram_parameter(
        "x", [B_PER, H, W, C], mybir.dt.float32, isOutput=False
    )
    y = nc.declare_dram_parameter(
        "y", [B_PER, 8, 8, CELL], mybir.dt.float32, isOutput=True
    )

    # One DMA per (sample, cell-row): reads the 512 KB input block
    # linearly and scatter-writes 2 KB chunks into the 8 output cells
    # (iteration order hc, j, chunk). The output linear offset of block
    # (b, i) equals the input linear offset. Jobs are interleaved
    # even/odd between the SP and ACT HWDGE queues so both queues walk
    # the same HBM neighborhood in lockstep (row-buffer co-locality).
    jobs = [(b * SAMPLE + i * ROWBLK) for b in range(B_PER) for i in range(8)]

    def emit(eng, offs, sem):
        for r in range(reps):
            for off in offs:
                in_ap = bass.AP(x, off, [[1, ROWBLK]])
                out_ap = bass.AP(y, off, [[CHUNK, HC], [CELL, 8], [1, CHUNK]])
                eng.dma_start(out=out_ap, in_=in_ap).then_inc(sem, 16)
        eng.wait_ge(sem, 16 * len(offs) * reps)

    with (
        nc.Block() as block,
        nc.semaphore("sem_sp") as sem_sp,
        nc.semaphore("sem_act") as sem_act,
    ):

        @block.sync
        def _(eng):
            emit(eng, jobs[0::2], sem_sp)

        @block.scalar
        def _(eng):
            emit(eng, jobs[1::2], sem_act)

    _cached[reps] = nc
    return nc


def kernel(inputs: np.ndarray) -> np.ndarray:
    nc = _build()
    inputs = np.ascontiguousarray(inputs, dtype=np.float32)
    in_maps = [
        {"x": inputs[k * B_PER : (k + 1) * B_PER]} for k in range(N_CORES)
    ]
    res = run_bass_kernel_spmd(nc, in_maps, list(range(N_CORES)))
    out = np.concatenate([res.results[k]["y"] for k in range(N_CORES)], axis=0)
    return out


# revision 3
# speedup vs baseline: 1.0161x; 1.0161x over previous
"""Chessboard rearrangement kernel for Trainium2.

Input  [64, 256, 256, 16] f32 -> output [64, 8, 8, 16384] f32 where
out[b, i, j] = inputs[b, i*32:(i+1)*32, j*32:(j+1)*32, :].reshape(-1).

Pure data movement (memory-bound): the permutation granule is one 2 KB
chunk (32 W-pixels x 16 channels).  Implemented as direct DRAM->DRAM
DMAs, one per (sample, cell-row) block (512 KB each, 64 per core).
Batch axis is sharded 8-way across NeuronCores (8 samples per core,
32 MiB in + 32 MiB out each, ~187 us HBM roofline at 358 GB/s/NC).

Measured design points (reps-slope steady-state, same-session A/B):
- SP takes the first half of the (sample, cell-row) jobs scatter-style
  (linear 512 KB read, scattered 2 KB writes); ACT takes the second
  half gather-style (scattered 2 KB reads within the block, linear
  512 KB write).  Mixing the two bus directions measured the most
  consistent top performer across sessions (~192-197 us steady-state)
  vs ~199-205 us for the all-scatter half-split baseline.
- Even/odd job interleaving between queues (HBM-window co-locality)
  measured equal-or-slightly-worse than the mixed-style split and did
  not stack with it (vB2); sessions drift by ~5-8 us on this shared
  device, all close variants re-measured head-to-head 20+ rounds.
- Rejected by measurement: 8x 4 MiB DMAs w/ globally-strided reads
  (223 us), through-SBUF staging, 3rd queue via Pool/SWDGE (+3-10 us),
  single_packet=True (+4 us), one queue only (~198 us), splitting each
  512 KB block between queues (no gain, higher variance).
- Per-DMA `.then_inc` is mandatory (walrus: "DGE must have sync info").
"""

import sys

sys.path.insert(0, "/opt/trn_rl_repo")

import numpy as np

import concourse.bass as bass
import concourse.mybir as mybir
from concourse.bass_utils import run_bass_kernel_spmd

B, H, W, C = 64, 256, 256, 16
N_CORES = 8
B_PER = B // N_CORES          # 8 samples per core
HC, WC = H // 8, W // 8       # 32, 32 per-cell spatial dims
CELL = HC * WC * C            # 16384 elements per output cell
SAMPLE = H * W * C            # 1048576 elements per sample
ROWBLK = HC * W * C           # 131072 elements per input cell-row block
CHUNK = WC * C                # 512 contiguous elements (2 KB)

_cached = {}


def _build(reps: int = 1):
    if reps in _cached:
        return _cached[reps]
    nc = bass.Bass()
    x = nc.declare_dram_parameter(
        "x", [B_PER, H, W, C], mybir.dt.float32, isOutput=False
    )
    y = nc.declare_dram_parameter(
        "y", [B_PER, 8, 8, CELL], mybir.dt.float32, isOutput=True
    )

    jobs = [(b * SAMPLE + i * ROWBLK) for b in range(B_PER) for i in range(8)]

    def emit(eng, offs, sem, style):
        for r in range(reps):
            for off in offs:
                if style == "scatter":
                    in_ap = bass.AP(x, off, [[1, ROWBLK]])
                    out_ap = bass.AP(
                        y, off, [[CHUNK, HC], [CELL, 8], [1, CHUNK]]
                    )
                else:
                    in_ap = bass.AP(
                        x, off, [[CHUNK, 8], [W * C, HC], [1, CHUNK]]
                    )
                    out_ap = bass.AP(
                        y, off, [[CELL, 8], [CHUNK, HC], [1, CHUNK]]
                    )
                eng.dma_start(out=out_ap, in_=in_ap).then_inc(sem, 16)
        eng.wait_ge(sem, 16 * len(offs) * reps)

    with (
        nc.Block() as block,
        nc.semaphore("s1") as s1,
        nc.semaphore("s2") as s2,
    ):

        @block.sync
        def _(eng):
            emit(eng, jobs[:32], s1, "scatter")

        @block.scalar
        def _(eng):
            emit(eng, jobs[32:], s2, "gather")

    _cached[reps] = nc
    return nc


def kernel(inputs: np.ndarray) -> np.ndarray:
    nc = _build()
    inputs = np.ascontiguousarray(inputs, dtype=np.float32)
    in_maps = [
        {"x": inputs[k * B_PER : (k + 1) * B_PER]} for k in range(N_CORES)
    ]
    res = run_bass_kernel_spmd(nc, in_maps, list(range(N_CORES)))
    out = np.concatenate([res.results[k]["y"] for k in range(N_CORES)], axis=0)
    return out
